# revision 7
# baseline (speedup 1.0000x reference)
"""Trainium2 Bass kernel for a dense transformer block (B=8, S=512, D=768, H=12, Fd=3072).

Sharding: pure data-parallel over batch — one batch element per NeuronCore,
weights replicated, no collectives.

Math layout trick: activations are kept feature-major ("T layout", [feat, seq])
through the attention pipeline so the TensorEngine (which contracts over the
partition dim) never needs an on-device transpose of the attention matrix:

  xT [768,512] (host-transposed)
  qT = wq.T @ xT, kT = wk.T @ xT          (T layout, per-partition bias via ACT)
  v  = xT.T @ wv (natural [t,d] layout, bias via K-augmentation)
  scoresT[t,s] = kT_h[:,tchunk].T @ qT_h
  expT = exp(scoresT + gmaskT[t,s])       (gmask = gauss bias + mask penalty,
                                           host-precomputed; no max-subtraction
                                           needed since scores are O(1))
  hT_aug[d+1, s] = [v_h | 1].T @ expT     (ones-column gives the softmax sums)
  hT = hT_aug[:64] * broadcast(1/sum)     (recip_approx_fast + rank-1 matmul bcast)
  proj = hT_all.T @ wproj (+ bias row)    -> natural [s, 768]; residual + LN1
  h1T  = PE-transpose(h1)                 (24 128x128 transposes)
  ff1T = w1.T @ h1T, gelu fused on PSUM->SBUF copy with per-partition b1
  ff2  = ff1T.T @ w2 (+ bias row)         -> natural; residual + LN2 -> out

Matmul inputs are bf16 (weights pre-cast on host), PSUM accumulation f32,
element-wise math f32. PSUM->SBUF moves go to ScalarE where DVE is the
phase bottleneck.
"""

import numpy as np
import ml_dtypes

import concourse.bass as bass
import concourse.mybir as mybir
import concourse.tile as tile
from concourse import bacc
from concourse import bass_utils
from concourse.masks import make_identity

BF = mybir.dt.bfloat16
F32 = mybir.dt.float32
AF = mybir.ActivationFunctionType
OP = mybir.AluOpType

B, S, D, H, Dh, Fd = 8, 512, 768, 12, 64, 3072
NCORES = 8
MASK_NEG = -30.0  # effectively -inf after exp given |scores+gauss| <~ 8
EPS = 1e-12

KD = D // 128      # 6  K-tiles over D
MS = S // 128      # 4  chunks over sequence
KF = Fd // 128     # 24 K-tiles over Fd
NT = 2             # N-tiles over D for natural-layout outputs (2 x 384)
ND = D // NT       # 384


def _trace(nc, io):
    with tile.TileContext(nc) as tc:
        _trace_body(nc, tc, io)


def _trace_body(nc, tc, io):
    from contextlib import ExitStack

    with ExitStack() as ctx:
        glob = ctx.enter_context(tc.tile_pool(name="glob", bufs=1))
        psum = ctx.enter_context(tc.tile_pool(name="psum", bufs=1, space="PSUM"))

        # ---- constants / small inputs ----
        ident_bf = glob.tile([128, 128], BF, tag="ident")
        make_identity(nc, ident_bf)
        ones_bf = glob.tile([1, 512], BF, tag="ones_bf")
        nc.vector.memset(ones_bf, 1.0)
        ones64_f = glob.tile([1, 64], F32, tag="ones64")
        nc.vector.memset(ones64_f, 1.0)
        eps_t = glob.tile([128, 1], F32, tag="eps")
        nc.vector.memset(eps_t, EPS)

        bq8_c = glob.tile([128, KD], F32, tag="bq8")
        nc.sync.dma_start(out=bq8_c, in_=io["bq8"].rearrange("(c p) -> p c", p=128))
        bk_c = glob.tile([128, KD], F32, tag="bk")
        nc.sync.dma_start(out=bk_c, in_=io["bk"].rearrange("(c p) -> p c", p=128))
        b1_c = glob.tile([128, KF], F32, tag="b1")
        nc.sync.dma_start(out=b1_c, in_=io["b1"].rearrange("(c p) -> p c", p=128))

        bv_r = glob.tile([1, D], BF, tag="bv")
        nc.sync.dma_start(out=bv_r, in_=io["bv_bf"].rearrange("(a n) -> a n", a=1))
        bproj_r = glob.tile([1, D], BF, tag="bproj")
        nc.sync.dma_start(out=bproj_r, in_=io["bproj_bf"].rearrange("(a n) -> a n", a=1))
        b2_r = glob.tile([1, D], BF, tag="b2")
        nc.sync.dma_start(out=b2_r, in_=io["b2_bf"].rearrange("(a n) -> a n", a=1))

        def bcast128(ap):
            return bass.AP(tensor=ap.tensor, offset=ap.offset, ap=[[0, 128]] + list(ap.ap))

        g1b = glob.tile([128, D], F32, tag="g1b")
        nc.gpsimd.dma_start(out=g1b, in_=bcast128(io["gamma1"]))
        be1b = glob.tile([128, D], F32, tag="be1b")
        nc.gpsimd.dma_start(out=be1b, in_=bcast128(io["beta1"]))
        g2b = glob.tile([128, D], F32, tag="g2b")
        nc.gpsimd.dma_start(out=g2b, in_=bcast128(io["gamma2"]))
        be2b = glob.tile([128, D], F32, tag="be2b")
        nc.gpsimd.dma_start(out=be2b, in_=bcast128(io["beta2"]))

        h1_sb = glob.tile([128, MS, D], F32, tag="h1")
        h1T_sb = glob.tile([128, KD, S], BF, tag="h1T")
        ff1T_sb = glob.tile([128, KF, S], BF, tag="ff1T")

        # ================= attention scope =================
        with tc.tile_pool(name="attn", bufs=1) as attnp:
            # weight DMAs first: the first matmuls need wq/wk + xT.
            w_qk = {}
            for wname in ("wq_bf", "wk_bf"):
                w_t = []
                for k in range(KD):
                    wt = attnp.tile([128, D], BF, tag="w6", bufs=8, name=f"{wname}{k}")
                    nc.sync.dma_start(out=wt, in_=io[wname].rearrange("(c p) n -> c p n", p=128)[k])
                    w_t.append(wt)
                w_qk[wname] = w_t
            xT_sb = attnp.tile([128, KD, S], BF, tag="xT")
            nc.sync.dma_start(out=xT_sb, in_=io["xT_bf"].rearrange("(c p) s -> p c s", p=128))

            qT_sb = attnp.tile([128, KD, S], BF, tag="qT")
            kT_sb = attnp.tile([128, KD, S], BF, tag="kT")
            v_sb = attnp.tile([128, MS, H, Dh + 1], BF, tag="v")
            nc.vector.memset(v_sb[:, :, :, Dh : Dh + 1], 1.0)
            hT_sb = attnp.tile([128, KD, S], BF, tag="hT")

            # --- qT / kT projections (T layout); bias (+0.125 scale for q) on ACT ---
            for wname, dst, bias_c, scale in (
                ("wq_bf", qT_sb, bq8_c, 0.125),
                ("wk_bf", kT_sb, bk_c, 1.0),
            ):
                w_t = w_qk[wname]
                for m in range(KD):
                    ps = psum.tile([128, 512], F32, tag="acc", bufs=2, name="ps_qk")
                    for k in range(KD):
                        nc.tensor.matmul(
                            ps, w_t[k][:, 128 * m : 128 * (m + 1)], xT_sb[:, k, :],
                            start=(k == 0), stop=(k == KD - 1),
                        )
                    nc.scalar.activation(
                        out=dst[:, m, :], in_=ps, func=AF.Identity,
                        bias=bias_c[:, m : m + 1], scale=scale,
                    )

            # --- v projection (natural layout, bias via K-augmentation) ---
            wv_t = []
            for k in range(KD):
                wt = attnp.tile([128, D], BF, tag="w6", bufs=8, name=f"wv{k}")
                nc.sync.dma_start(out=wt, in_=io["wv_bf"].rearrange("(c p) n -> c p n", p=128)[k])
                wv_t.append(wt)
            gauss_sb = attnp.tile([128, MS, S], F32, tag="gauss")
            nc.sync.dma_start(out=gauss_sb, in_=io["gmask"].rearrange("(c p) s -> p c s", p=128))
            x_sb = glob.tile([128, MS, D], F32, tag="x")
            nc.sync.dma_start(out=x_sb, in_=io["x"].rearrange("(c p) n -> p c n", p=128))

            for c in range(MS):
                for n in range(NT):
                    ps = psum.tile([128, ND], F32, tag="acc", bufs=2, name="ps_v")
                    for k in range(KD):
                        nc.tensor.matmul(
                            ps, xT_sb[:, k, 128 * c : 128 * (c + 1)],
                            wv_t[k][:, ND * n : ND * (n + 1)],
                            start=(k == 0), stop=False,
                        )
                    nc.tensor.matmul(
                        ps, ones_bf[:, 0:128], bv_r[:, ND * n : ND * (n + 1)],
                        start=False, stop=True,
                    )
                    nc.scalar.copy(
                        out=v_sb[:, c, 6 * n : 6 * (n + 1), 0:Dh],
                        in_=ps.rearrange("p (h d) -> p h d", d=Dh),
                    )

            # --- per-head attention ---
            for h in range(H):
                th, off = h // 2, (h % 2) * 64
                qh = qT_sb[off : off + 64, th, :]
                kh = kT_sb[off : off + 64, th, :]
                exp_tiles = []
                for half in range(2):
                    ps_sc = psum.tile([128, 2, 512], F32, tag="sc2", bufs=2, name="ps_sc")
                    for j in range(2):
                        c = 2 * half + j
                        nc.tensor.matmul(
                            ps_sc[:, j, :], kh[:, 128 * c : 128 * (c + 1)], qh,
                            start=True, stop=True,
                        )
                    tmp = attnp.tile([128, 2, 512], F32, tag="exptmp", bufs=2, name="tmp")
                    nc.vector.tensor_tensor(
                        out=tmp, in0=ps_sc,
                        in1=gauss_sb[:, 2 * half : 2 * half + 2, :], op=OP.add,
                    )
                    ex = attnp.tile([128, 2, 512], BF, tag="exp", bufs=3, name="ex")
                    nc.scalar.activation(out=ex, in_=tmp, func=AF.Exp)
                    exp_tiles.append(ex)
                ps_h = psum.tile([Dh + 1, 512], F32, tag="hT", bufs=1, name="ps_h")
                for c in range(MS):
                    nc.tensor.matmul(
                        ps_h, v_sb[:, c, h, :], exp_tiles[c // 2][:, c % 2, :],
                        start=(c == 0), stop=(c == MS - 1),
                    )
                srow = attnp.tile([1, 512], F32, tag="srow", bufs=2, name="srow")
                nc.scalar.copy(out=srow, in_=ps_h[Dh : Dh + 1, :])
                rec = attnp.tile([1, 512], F32, tag="rec", bufs=2, name="rec")
                nc.vector.reciprocal_approx_fast(out=rec, in_=srow)
                ps_b = psum.tile([64, 512], F32, tag="bc", bufs=1, name="ps_b")
                nc.tensor.matmul(ps_b, ones64_f, rec, start=True, stop=True)
                bca = attnp.tile([64, 512], F32, tag="bca", bufs=2, name="bca")
                nc.scalar.copy(bca, ps_b)
                nc.vector.tensor_tensor(
                    out=hT_sb[off : off + 64, th, :], in0=ps_h[0:64, :], in1=bca, op=OP.mult
                )

            # --- proj + residual + LN1 (+ h1 transpose) ---
            wp_t = []
            for k in range(KD):
                wt = attnp.tile([128, D], BF, tag="w6", bufs=8, name=f"wp{k}")
                nc.sync.dma_start(out=wt, in_=io["wproj_bf"].rearrange("(c p) n -> c p n", p=128)[k])
                wp_t.append(wt)
            for m in range(MS):
                pss = []
                for n in range(NT):
                    ps = psum.tile([128, ND], F32, tag="acc", bufs=2, name="ps_pr")
                    for k in range(KD):
                        nc.tensor.matmul(
                            ps, hT_sb[:, k, 128 * m : 128 * (m + 1)],
                            wp_t[k][:, ND * n : ND * (n + 1)],
                            start=(k == 0), stop=False,
                        )
                    nc.tensor.matmul(
                        ps, ones_bf[:, 0:128], bproj_r[:, ND * n : ND * (n + 1)],
                        start=False, stop=True,
                    )
                    pss.append(ps)
                row = glob.tile([128, D], F32, tag="rowtmp", bufs=3, name="row")
                for n in range(NT):
                    nc.vector.tensor_tensor(
                        out=row[:, ND * n : ND * (n + 1)], in0=pss[n],
                        in1=x_sb[:, m, ND * n : ND * (n + 1)], op=OP.add,
                    )
                _layernorm(nc, glob, row, g1b, be1b, eps_t, h1_sb[:, m, :])
                h1bf = glob.tile([128, D], BF, tag="h1bf", bufs=2, name="h1bf")
                nc.vector.tensor_copy(h1bf, h1_sb[:, m, :])
                for f in range(KD):
                    ps_t = psum.tile([128, 128], BF, tag="acc", bufs=2, name="ps_t")
                    nc.tensor.transpose(ps_t, h1bf[:, 128 * f : 128 * (f + 1)], ident_bf)
                    nc.scalar.copy(out=h1T_sb[:, f, 128 * m : 128 * (m + 1)], in_=ps_t)

        # ================= FFN scope =================
        with tc.tile_pool(name="ffn", bufs=1) as ffnp:
            w1_t = []
            for k in range(KD):
                wt = ffnp.tile([128, Fd], BF, tag="w1", bufs=6, name=f"w1_{k}")
                nc.sync.dma_start(out=wt, in_=io["w1_bf"].rearrange("(c p) n -> c p n", p=128)[k])
                w1_t.append(wt)
            for fm in range(KF):
                ps = psum.tile([128, 512], F32, tag="acc", bufs=2, name="ps_f1")
                for k in range(KD):
                    nc.tensor.matmul(
                        ps, w1_t[k][:, 128 * fm : 128 * (fm + 1)], h1T_sb[:, k, :],
                        start=(k == 0), stop=(k == KD - 1),
                    )
                nc.scalar.activation(
                    out=ff1T_sb[:, fm, :], in_=ps, func=AF.Gelu,
                    bias=b1_c[:, fm : fm + 1], scale=1.0,
                )

            w2_t = []
            for k in range(KF):
                wt = ffnp.tile([128, D], BF, tag="w2", bufs=KF, name=f"w2_{k}")
                nc.sync.dma_start(out=wt, in_=io["w2_bf"].rearrange("(c p) n -> c p n", p=128)[k])
                w2_t.append(wt)
            for m in range(MS):
                pss = []
                for n in range(NT):
                    ps = psum.tile([128, ND], F32, tag="acc", bufs=2, name="ps_f2")
                    for k in range(KF):
                        nc.tensor.matmul(
                            ps, ff1T_sb[:, k, 128 * m : 128 * (m + 1)],
                            w2_t[k][:, ND * n : ND * (n + 1)],
                            start=(k == 0), stop=False,
                        )
                    nc.tensor.matmul(
                        ps, ones_bf[:, 0:128], b2_r[:, ND * n : ND * (n + 1)],
                        start=False, stop=True,
                    )
                    pss.append(ps)
                row = glob.tile([128, D], F32, tag="rowtmp", bufs=3, name="row2")
                for n in range(NT):
                    nc.vector.tensor_tensor(
                        out=row[:, ND * n : ND * (n + 1)], in0=pss[n],
                        in1=h1_sb[:, m, ND * n : ND * (n + 1)], op=OP.add,
                    )
                outrow = glob.tile([128, D], F32, tag="outrow", bufs=2, name="outrow")
                _layernorm(nc, glob, row, g2b, be2b, eps_t, outrow)
                nc.sync.dma_start(
                    out=io["out"][128 * m : 128 * (m + 1), :], in_=outrow
                )


def _layernorm(nc, pool, row, gamma_b, beta_b, eps_t, out_ap):
    st = pool.tile([128, 3, 6], F32, tag="st", bufs=4, name="st")
    for g in range(3):
        nc.vector.bn_stats(out=st[:, g, :], in_=row[:, 256 * g : 256 * (g + 1)])
    mv = pool.tile([128, 2], F32, tag="mv", bufs=4, name="mv")
    nc.vector.bn_aggr(out=mv, in_=st)
    sd = pool.tile([128, 1], F32, tag="sd", bufs=4, name="sd")
    nc.scalar.activation(out=sd, in_=mv[:, 1:2], func=AF.Sqrt, bias=eps_t, scale=1.0)
    rs = pool.tile([128, 1], F32, tag="rs", bufs=4, name="rs")
    nc.vector.reciprocal(rs, sd)
    # in-place: row = (row - mean) * gamma ; out = row * rstd + beta
    nc.vector.scalar_tensor_tensor(
        out=row, in0=row, scalar=mv[:, 0:1], in1=gamma_b, op0=OP.subtract, op1=OP.mult
    )
    nc.vector.scalar_tensor_tensor(
        out=out_ap, in0=row, scalar=rs, in1=beta_b, op0=OP.mult, op1=OP.add
    )


_SPECS = [
    # (name, shape, dtype)
    ("x", [S, D], F32),
    ("xT_bf", [D, S], BF),
    ("gmask", [S, S], F32),
    ("wq_bf", [D, D], BF),
    ("wk_bf", [D, D], BF),
    ("wv_bf", [D, D], BF),
    ("wproj_bf", [D, D], BF),
    ("w1_bf", [D, Fd], BF),
    ("w2_bf", [Fd, D], BF),
    ("bq8", [D], F32),
    ("bk", [D], F32),
    ("bv_bf", [D], BF),
    ("bproj_bf", [D], BF),
    ("b1", [Fd], F32),
    ("b2_bf", [D], BF),
    ("gamma1", [D], F32),
    ("beta1", [D], F32),
    ("gamma2", [D], F32),
    ("beta2", [D], F32),
]

_BUILT = {}


def _build():
    if "nc" in _BUILT:
        return _BUILT["nc"]
    nc = bacc.Bacc("TRN2", target_bir_lowering=False, debug=False,
                   enable_asserts=False, num_devices=NCORES)
    io = {}
    for name, shape, dt in _SPECS:
        io[name] = nc.dram_tensor(name, shape, dt, kind="ExternalInput").ap()
    io["out"] = nc.dram_tensor("out", [S, D], F32, kind="ExternalOutput").ap()
    _trace(nc, io)
    nc.compile()
    _BUILT["nc"] = nc
    return nc


def _host_prep(inputs):
    bf = ml_dtypes.bfloat16
    f32 = np.float32
    x = np.asarray(inputs["x"], f32)
    mask = np.asarray(inputs["mask"])

    idx = np.arange(S, dtype=np.float64)
    dd = idx[None, :] - idx[:, None]
    sc = -0.5 * dd * dd
    sc -= sc.max(axis=-1, keepdims=True)
    e = np.exp(sc)
    gauss = (e / e.sum(axis=-1, keepdims=True)).astype(f32)  # [i=s, j=t]
    gaussT = np.ascontiguousarray(gauss.T)

    shared = {
        "wq_bf": np.asarray(inputs["wq"], f32).astype(bf),
        "wk_bf": np.asarray(inputs["wk"], f32).astype(bf),
        "wv_bf": np.asarray(inputs["wv"], f32).astype(bf),
        "wproj_bf": np.asarray(inputs["w_proj"], f32).astype(bf),
        "w1_bf": np.asarray(inputs["w1"], f32).astype(bf),
        "w2_bf": np.asarray(inputs["w2"], f32).astype(bf),
        "bq8": np.asarray(inputs["bq"], f32) * np.float32(0.125),
        "bk": np.asarray(inputs["bk"], f32),
        "bv_bf": np.asarray(inputs["bv"], f32).astype(bf),
        "bproj_bf": np.asarray(inputs["b_proj"], f32).astype(bf),
        "b1": np.asarray(inputs["b1"], f32),
        "b2_bf": np.asarray(inputs["b2"], f32).astype(bf),
        "gamma1": np.asarray(inputs["gamma1"], f32),
        "beta1": np.asarray(inputs["beta1"], f32),
        "gamma2": np.asarray(inputs["gamma2"], f32),
        "beta2": np.asarray(inputs["beta2"], f32),
    }
    in_maps = []
    for b in range(NCORES):
        m = dict(shared)
        m["x"] = np.ascontiguousarray(x[b])
        m["xT_bf"] = np.ascontiguousarray(x[b].T).astype(bf)
        m["gmask"] = gaussT + (MASK_NEG * (1.0 - mask[b].astype(f32)))[:, None]
        in_maps.append(m)
    return in_maps


def _run(inputs, trace=False, trace_cores=None):
    nc = _build()
    in_maps = _host_prep(inputs)
    res = bass_utils.run_bass_kernel_spmd(
        nc, in_maps, core_ids=list(range(NCORES)), trace=trace,
        trace_cores=trace_cores,
    )
    out = np.stack([np.asarray(res.results[b]["out"]) for b in range(NCORES)])
    return out.astype(np.float32), res


def kernel(**inputs):
    return _run(inputs)[0]


# revision 8
# speedup vs baseline: 1.0159x; 1.0159x over previous
"""Trainium2 Bass kernel for a dense transformer block (B=8, S=512, D=768, H=12, Fd=3072).

Sharding: pure data-parallel over batch — one batch element per NeuronCore,
weights replicated, no collectives.

Math layout trick: activations are kept feature-major ("T layout", [feat, seq])
through the attention pipeline so the TensorEngine (which contracts over the
partition dim) never needs an on-device transpose of the attention matrix:

  xT [768,512] (host-transposed)
  qT = wq.T @ xT, kT = wk.T @ xT          (T layout, per-partition bias via ACT)
  v  = xT.T @ wv (natural [t,d] layout, bias via K-augmentation)
  scoresT[t,s] = kT_h[:,tchunk].T @ qT_h
  expT = exp(scoresT + gmaskT[t,s])       (gmask = gauss bias + mask penalty,
                                           host-precomputed; no max-subtraction
                                           needed since scores are O(1))
  hT_aug[d+1, s] = [v_h | 1].T @ expT     (ones-column gives the softmax sums)
  hT = hT_aug[:64] * broadcast(1/sum)     (recip_approx_fast + rank-1 matmul bcast)
  proj = hT_all.T @ wproj (+ bias row)    -> natural [s, 768]; residual + LN1
  h1T  = PE-transpose(h1)                 (24 128x128 transposes)
  ff1T = w1.T @ h1T, gelu fused on PSUM->SBUF copy with per-partition b1
  ff2  = ff1T.T @ w2 (+ bias row)         -> natural; residual + LN2 -> out

Matmul inputs are bf16 (weights pre-cast on host), PSUM accumulation f32,
element-wise math f32. PSUM->SBUF moves go to ScalarE where DVE is the
phase bottleneck.
"""

import numpy as np
import ml_dtypes

import concourse.bass as bass
import concourse.mybir as mybir
import concourse.tile as tile
from concourse import bacc
from concourse import bass_utils
from concourse.masks import make_identity

BF = mybir.dt.bfloat16
F32 = mybir.dt.float32
AF = mybir.ActivationFunctionType
OP = mybir.AluOpType

B, S, D, H, Dh, Fd = 8, 512, 768, 12, 64, 3072
NCORES = 8
MASK_NEG = -30.0  # effectively -inf after exp given |scores+gauss| <~ 8
EPS = 1e-12

KD = D // 128      # 6  K-tiles over D
MS = S // 128      # 4  chunks over sequence
KF = Fd // 128     # 24 K-tiles over Fd
NT = 2             # N-tiles over D for natural-layout outputs (2 x 384)
ND = D // NT       # 384


def _trace(nc, io):
    with tile.TileContext(nc) as tc:
        _trace_body(nc, tc, io)


def _trace_body(nc, tc, io):
    from contextlib import ExitStack

    with ExitStack() as ctx:
        glob = ctx.enter_context(tc.tile_pool(name="glob", bufs=1))
        psum = ctx.enter_context(tc.tile_pool(name="psum", bufs=1, space="PSUM"))

        # ---- constants / small inputs ----
        ident_bf = glob.tile([128, 128], BF, tag="ident")
        make_identity(nc, ident_bf)
        ones_bf = glob.tile([1, 512], BF, tag="ones_bf")
        nc.vector.memset(ones_bf, 1.0)
        ones64_f = glob.tile([1, 64], F32, tag="ones64")
        nc.vector.memset(ones64_f, 1.0)
        eps_t = glob.tile([128, 1], F32, tag="eps")
        nc.vector.memset(eps_t, EPS)

        bq8_c = glob.tile([128, KD], F32, tag="bq8")
        nc.sync.dma_start(out=bq8_c, in_=io["bq8"].rearrange("(c p) -> p c", p=128))
        bk_c = glob.tile([128, KD], F32, tag="bk")
        nc.sync.dma_start(out=bk_c, in_=io["bk"].rearrange("(c p) -> p c", p=128))
        b1_c = glob.tile([128, KF], F32, tag="b1")
        nc.sync.dma_start(out=b1_c, in_=io["b1"].rearrange("(c p) -> p c", p=128))

        bv_r = glob.tile([1, D], BF, tag="bv")
        nc.sync.dma_start(out=bv_r, in_=io["bv_bf"].rearrange("(a n) -> a n", a=1))
        bproj_r = glob.tile([1, D], BF, tag="bproj")
        nc.sync.dma_start(out=bproj_r, in_=io["bproj_bf"].rearrange("(a n) -> a n", a=1))
        b2_r = glob.tile([1, D], BF, tag="b2")
        nc.sync.dma_start(out=b2_r, in_=io["b2_bf"].rearrange("(a n) -> a n", a=1))

        def bcast128(ap):
            return bass.AP(tensor=ap.tensor, offset=ap.offset, ap=[[0, 128]] + list(ap.ap))

        g1b = glob.tile([128, D], F32, tag="g1b")
        nc.gpsimd.dma_start(out=g1b, in_=bcast128(io["gamma1"]))
        be1b = glob.tile([128, D], F32, tag="be1b")
        nc.gpsimd.dma_start(out=be1b, in_=bcast128(io["beta1"]))
        g2b = glob.tile([128, D], F32, tag="g2b")
        nc.gpsimd.dma_start(out=g2b, in_=bcast128(io["gamma2"]))
        be2b = glob.tile([128, D], F32, tag="be2b")
        nc.gpsimd.dma_start(out=be2b, in_=bcast128(io["beta2"]))

        h1_sb = glob.tile([128, MS, D], F32, tag="h1")
        h1T_sb = glob.tile([128, KD, S], BF, tag="h1T")
        ff1T_sb = glob.tile([128, KF, S], BF, tag="ff1T")

        # ================= attention scope =================
        with tc.tile_pool(name="attn", bufs=1) as attnp:
            # weight DMAs first: the first matmuls need wq/wk + xT.
            w_qk = {}
            for wname in ("wq_bf", "wk_bf"):
                w_t = []
                for k in range(KD):
                    wt = attnp.tile([128, D], BF, tag="w6", bufs=8, name=f"{wname}{k}")
                    nc.sync.dma_start(out=wt, in_=io[wname].rearrange("(c p) n -> c p n", p=128)[k])
                    w_t.append(wt)
                w_qk[wname] = w_t
            xT_sb = attnp.tile([128, KD, S], BF, tag="xT")
            for k in range(KD):
                nc.sync.dma_start(out=xT_sb[:, k, :], in_=io["xT_bf"].rearrange("(c p) s -> c p s", p=128)[k])

            qT_sb = attnp.tile([128, KD, S], BF, tag="qT")
            kT_sb = attnp.tile([128, KD, S], BF, tag="kT")
            v_sb = attnp.tile([128, MS, H, Dh + 1], BF, tag="v")
            nc.vector.memset(v_sb[:, :, :, Dh : Dh + 1], 1.0)
            hT_sb = attnp.tile([128, KD, S], BF, tag="hT")

            # --- qT / kT projections (T layout); bias (+0.125 scale for q) on ACT ---
            for wname, dst, bias_c, scale in (
                ("wq_bf", qT_sb, bq8_c, 0.125),
                ("wk_bf", kT_sb, bk_c, 1.0),
            ):
                w_t = w_qk[wname]
                for m in range(KD):
                    ps = psum.tile([128, 512], F32, tag="acc", bufs=2, name="ps_qk")
                    for k in range(KD):
                        nc.tensor.matmul(
                            ps, w_t[k][:, 128 * m : 128 * (m + 1)], xT_sb[:, k, :],
                            start=(k == 0), stop=(k == KD - 1),
                        )
                    nc.scalar.activation(
                        out=dst[:, m, :], in_=ps, func=AF.Identity,
                        bias=bias_c[:, m : m + 1], scale=scale,
                    )

            # --- v projection (natural layout, bias via K-augmentation) ---
            wv_t = []
            for k in range(KD):
                wt = attnp.tile([128, D], BF, tag="w6", bufs=8, name=f"wv{k}")
                nc.sync.dma_start(out=wt, in_=io["wv_bf"].rearrange("(c p) n -> c p n", p=128)[k])
                wv_t.append(wt)
            gauss_sb = attnp.tile([128, MS, S], BF, tag="gauss")
            for c in range(MS):
                nc.sync.dma_start(out=gauss_sb[:, c, :], in_=io["gexp"].rearrange("(c p) s -> c p s", p=128)[c])
            x_sb = glob.tile([128, MS, D], F32, tag="x")
            for c in range(MS):
                nc.sync.dma_start(out=x_sb[:, c, :], in_=io["x"].rearrange("(c p) n -> c p n", p=128)[c])

            for c in range(MS):
                for n in range(NT):
                    ps = psum.tile([128, ND], F32, tag="acc", bufs=2, name="ps_v")
                    for k in range(KD):
                        nc.tensor.matmul(
                            ps, xT_sb[:, k, 128 * c : 128 * (c + 1)],
                            wv_t[k][:, ND * n : ND * (n + 1)],
                            start=(k == 0), stop=False,
                        )
                    nc.tensor.matmul(
                        ps, ones_bf[:, 0:128], bv_r[:, ND * n : ND * (n + 1)],
                        start=False, stop=True,
                    )
                    nc.scalar.copy(
                        out=v_sb[:, c, 6 * n : 6 * (n + 1), 0:Dh],
                        in_=ps.rearrange("p (h d) -> p h d", d=Dh),
                    )

            # --- per-head attention ---
            for h in range(H):
                th, off = h // 2, (h % 2) * 64
                qh = qT_sb[off : off + 64, th, :]
                kh = kT_sb[off : off + 64, th, :]
                exp_tiles = []
                for half in range(2):
                    ps_sc = psum.tile([128, 2, 512], F32, tag="sc2", bufs=2, name="ps_sc")
                    for j in range(2):
                        c = 2 * half + j
                        nc.tensor.matmul(
                            ps_sc[:, j, :], kh[:, 128 * c : 128 * (c + 1)], qh,
                            start=True, stop=True,
                        )
                    exraw = attnp.tile([128, 2, 512], BF, tag="exraw", bufs=4, name="exraw")
                    nc.scalar.activation(out=exraw, in_=ps_sc, func=AF.Exp)
                    ex = attnp.tile([128, 2, 512], BF, tag="exp", bufs=4, name="ex")
                    nc.vector.tensor_tensor(
                        out=ex, in0=exraw,
                        in1=gauss_sb[:, 2 * half : 2 * half + 2, :], op=OP.mult,
                    )
                    exp_tiles.append(ex)
                ps_h = psum.tile([Dh + 1, 512], F32, tag="hT", bufs=1, name="ps_h")
                for c in range(MS):
                    nc.tensor.matmul(
                        ps_h, v_sb[:, c, h, :], exp_tiles[c // 2][:, c % 2, :],
                        start=(c == 0), stop=(c == MS - 1),
                    )
                srow = attnp.tile([1, 512], F32, tag="srow", bufs=3, name="srow")
                nc.scalar.copy(out=srow, in_=ps_h[Dh : Dh + 1, :])
                rec = attnp.tile([1, 512], F32, tag="rec", bufs=3, name="rec")
                nc.vector.reciprocal_approx_fast(out=rec, in_=srow)
                ps_b = psum.tile([64, 512], F32, tag="bc", bufs=1, name="ps_b")
                nc.tensor.matmul(ps_b, ones64_f, rec, start=True, stop=True)
                bca = attnp.tile([64, 512], F32, tag="bca", bufs=3, name="bca")
                nc.vector.tensor_copy(bca, ps_b)
                nc.vector.tensor_tensor(
                    out=hT_sb[off : off + 64, th, :], in0=ps_h[0:64, :], in1=bca, op=OP.mult
                )

            # --- proj + residual + LN1 (+ h1 transpose) ---
            wp_t = []
            for k in range(KD):
                wt = attnp.tile([128, D], BF, tag="w6", bufs=8, name=f"wp{k}")
                nc.sync.dma_start(out=wt, in_=io["wproj_bf"].rearrange("(c p) n -> c p n", p=128)[k])
                wp_t.append(wt)
            for m in range(MS):
                pss = []
                for n in range(NT):
                    ps = psum.tile([128, ND], F32, tag="acc", bufs=2, name="ps_pr")
                    for k in range(KD):
                        nc.tensor.matmul(
                            ps, hT_sb[:, k, 128 * m : 128 * (m + 1)],
                            wp_t[k][:, ND * n : ND * (n + 1)],
                            start=(k == 0), stop=False,
                        )
                    nc.tensor.matmul(
                        ps, ones_bf[:, 0:128], bproj_r[:, ND * n : ND * (n + 1)],
                        start=False, stop=True,
                    )
                    pss.append(ps)
                row = glob.tile([128, D], F32, tag="rowtmp", bufs=3, name="row")
                for n in range(NT):
                    nc.vector.tensor_tensor(
                        out=row[:, ND * n : ND * (n + 1)], in0=pss[n],
                        in1=x_sb[:, m, ND * n : ND * (n + 1)], op=OP.add,
                    )
                _layernorm(nc, glob, row, g1b, be1b, eps_t, h1_sb[:, m, :])
                h1bf = glob.tile([128, D], BF, tag="h1bf", bufs=2, name="h1bf")
                nc.vector.tensor_copy(h1bf, h1_sb[:, m, :])
                for f in range(KD):
                    ps_t = psum.tile([128, 128], BF, tag="acc", bufs=2, name="ps_t")
                    nc.tensor.transpose(ps_t, h1bf[:, 128 * f : 128 * (f + 1)], ident_bf)
                    nc.scalar.copy(out=h1T_sb[:, f, 128 * m : 128 * (m + 1)], in_=ps_t)

        # ================= FFN scope =================
        with tc.tile_pool(name="ffn", bufs=1) as ffnp:
            w1_t = []
            for k in range(KD):
                wt = ffnp.tile([128, Fd], BF, tag="w1", bufs=6, name=f"w1_{k}")
                nc.sync.dma_start(out=wt, in_=io["w1_bf"].rearrange("(c p) n -> c p n", p=128)[k])
                w1_t.append(wt)
            for fm in range(KF):
                ps = psum.tile([128, 512], F32, tag="acc", bufs=2, name="ps_f1")
                for k in range(KD):
                    nc.tensor.matmul(
                        ps, w1_t[k][:, 128 * fm : 128 * (fm + 1)], h1T_sb[:, k, :],
                        start=(k == 0), stop=(k == KD - 1),
                    )
                nc.scalar.activation(
                    out=ff1T_sb[:, fm, :], in_=ps, func=AF.Gelu,
                    bias=b1_c[:, fm : fm + 1], scale=1.0,
                )

            w2_t = []
            for k in range(KF):
                wt = ffnp.tile([128, D], BF, tag="w2", bufs=KF, name=f"w2_{k}")
                nc.sync.dma_start(out=wt, in_=io["w2_bf"].rearrange("(c p) n -> c p n", p=128)[k])
                w2_t.append(wt)
            for m in range(MS):
                pss = []
                for n in range(NT):
                    ps = psum.tile([128, ND], F32, tag="acc", bufs=2, name="ps_f2")
                    for k in range(KF):
                        nc.tensor.matmul(
                            ps, ff1T_sb[:, k, 128 * m : 128 * (m + 1)],
                            w2_t[k][:, ND * n : ND * (n + 1)],
                            start=(k == 0), stop=False,
                        )
                    nc.tensor.matmul(
                        ps, ones_bf[:, 0:128], b2_r[:, ND * n : ND * (n + 1)],
                        start=False, stop=True,
                    )
                    pss.append(ps)
                row = glob.tile([128, D], F32, tag="rowtmp", bufs=3, name="row2")
                for n in range(NT):
                    nc.vector.tensor_tensor(
                        out=row[:, ND * n : ND * (n + 1)], in0=pss[n],
                        in1=h1_sb[:, m, ND * n : ND * (n + 1)], op=OP.add,
                    )
                outrow = glob.tile([128, D], F32, tag="outrow", bufs=2, name="outrow")
                _layernorm(nc, glob, row, g2b, be2b, eps_t, outrow)
                nc.sync.dma_start(
                    out=io["out"][128 * m : 128 * (m + 1), :], in_=outrow
                )


def _layernorm(nc, pool, row, gamma_b, beta_b, eps_t, out_ap):
    st = pool.tile([128, 3, 6], F32, tag="st", bufs=4, name="st")
    for g in range(3):
        nc.vector.bn_stats(out=st[:, g, :], in_=row[:, 256 * g : 256 * (g + 1)])
    mv = pool.tile([128, 2], F32, tag="mv", bufs=4, name="mv")
    nc.vector.bn_aggr(out=mv, in_=st)
    sd = pool.tile([128, 1], F32, tag="sd", bufs=4, name="sd")
    nc.scalar.activation(out=sd, in_=mv[:, 1:2], func=AF.Sqrt, bias=eps_t, scale=1.0)
    rs = pool.tile([128, 1], F32, tag="rs", bufs=4, name="rs")
    nc.vector.reciprocal(rs, sd)
    # in-place: row = (row - mean) * gamma ; out = row * rstd + beta
    nc.vector.scalar_tensor_tensor(
        out=row, in0=row, scalar=mv[:, 0:1], in1=gamma_b, op0=OP.subtract, op1=OP.mult
    )
    nc.vector.scalar_tensor_tensor(
        out=out_ap, in0=row, scalar=rs, in1=beta_b, op0=OP.mult, op1=OP.add
    )


_SPECS = [
    # (name, shape, dtype)
    ("x", [S, D], F32),
    ("xT_bf", [D, S], BF),
    ("gexp", [S, S], BF),
    ("wq_bf", [D, D], BF),
    ("wk_bf", [D, D], BF),
    ("wv_bf", [D, D], BF),
    ("wproj_bf", [D, D], BF),
    ("w1_bf", [D, Fd], BF),
    ("w2_bf", [Fd, D], BF),
    ("bq8", [D], F32),
    ("bk", [D], F32),
    ("bv_bf", [D], BF),
    ("bproj_bf", [D], BF),
    ("b1", [Fd], F32),
    ("b2_bf", [D], BF),
    ("gamma1", [D], F32),
    ("beta1", [D], F32),
    ("gamma2", [D], F32),
    ("beta2", [D], F32),
]

_BUILT = {}


def _build():
    if "nc" in _BUILT:
        return _BUILT["nc"]
    nc = bacc.Bacc("TRN2", target_bir_lowering=False, debug=False,
                   enable_asserts=False, num_devices=NCORES)
    io = {}
    for name, shape, dt in _SPECS:
        io[name] = nc.dram_tensor(name, shape, dt, kind="ExternalInput").ap()
    io["out"] = nc.dram_tensor("out", [S, D], F32, kind="ExternalOutput").ap()
    _trace(nc, io)
    nc.compile()
    _BUILT["nc"] = nc
    return nc


def _host_prep(inputs):
    bf = ml_dtypes.bfloat16
    f32 = np.float32
    x = np.asarray(inputs["x"], f32)
    mask = np.asarray(inputs["mask"])

    idx = np.arange(S, dtype=np.float64)
    dd = idx[None, :] - idx[:, None]
    sc = -0.5 * dd * dd
    sc -= sc.max(axis=-1, keepdims=True)
    e = np.exp(sc)
    gauss = (e / e.sum(axis=-1, keepdims=True)).astype(f32)  # [i=s, j=t]
    gaussT = np.ascontiguousarray(gauss.T)

    shared = {
        "wq_bf": np.asarray(inputs["wq"], f32).astype(bf),
        "wk_bf": np.asarray(inputs["wk"], f32).astype(bf),
        "wv_bf": np.asarray(inputs["wv"], f32).astype(bf),
        "wproj_bf": np.asarray(inputs["w_proj"], f32).astype(bf),
        "w1_bf": np.asarray(inputs["w1"], f32).astype(bf),
        "w2_bf": np.asarray(inputs["w2"], f32).astype(bf),
        "bq8": np.asarray(inputs["bq"], f32) * np.float32(0.125),
        "bk": np.asarray(inputs["bk"], f32),
        "bv_bf": np.asarray(inputs["bv"], f32).astype(bf),
        "bproj_bf": np.asarray(inputs["b_proj"], f32).astype(bf),
        "b1": np.asarray(inputs["b1"], f32),
        "b2_bf": np.asarray(inputs["b2"], f32).astype(bf),
        "gamma1": np.asarray(inputs["gamma1"], f32),
        "beta1": np.asarray(inputs["beta1"], f32),
        "gamma2": np.asarray(inputs["gamma2"], f32),
        "beta2": np.asarray(inputs["beta2"], f32),
    }
    in_maps = []
    for b in range(NCORES):
        m = dict(shared)
        m["x"] = np.ascontiguousarray(x[b])
        m["xT_bf"] = np.ascontiguousarray(x[b].T).astype(bf)
        gm = gaussT + (MASK_NEG * (1.0 - mask[b].astype(f32)))[:, None]
        m["gexp"] = np.exp(gm).astype(bf)
        in_maps.append(m)
    return in_maps


def _run(inputs, trace=False, trace_cores=None):
    nc = _build()
    in_maps = _host_prep(inputs)
    res = bass_utils.run_bass_kernel_spmd(
        nc, in_maps, core_ids=list(range(NCORES)), trace=trace,
        trace_cores=trace_cores,
    )
    out = np.stack([np.asarray(res.results[b]["out"]) for b in range(NCORES)])
    return out.astype(np.float32), res


def kernel(**inputs):
    return _run(inputs)[0]


# revision 9
# speedup vs baseline: 1.1150x; 1.0975x over previous
"""Trainium2 Bass kernel for a dense transformer block (B=8, S=512, D=768, H=12, Fd=3072).

Sharding: pure data-parallel over batch — one batch element per NeuronCore,
weights replicated, no collectives.

Math layout trick: activations are kept feature-major ("T layout", [feat, seq])
through the attention pipeline so the TensorEngine (which contracts over the
partition dim) never needs an on-device transpose of the attention matrix:

  xT [768,512] (host-transposed)
  qT = wq.T @ xT, kT = wk.T @ xT          (T layout, per-partition bias via ACT)
  v  = xT.T @ wv (natural [t,d] layout, bias via K-augmentation)
  scoresT[t,s] = kT_h[:,tchunk].T @ qT_h
  expT = exp(scoresT + gmaskT[t,s])       (gmask = gauss bias + mask penalty,
                                           host-precomputed; no max-subtraction
                                           needed since scores are O(1))
  hT_aug[d+1, s] = [v_h | 1].T @ expT     (ones-column gives the softmax sums)
  hT = hT_aug[:64] * broadcast(1/sum)     (recip_approx_fast + rank-1 matmul bcast)
  proj = hT_all.T @ wproj (+ bias row)    -> natural [s, 768]; residual + LN1
  h1T  = PE-transpose(h1)                 (24 128x128 transposes)
  ff1T = w1.T @ h1T, gelu fused on PSUM->SBUF copy with per-partition b1
  ff2  = ff1T.T @ w2 (+ bias row)         -> natural; residual + LN2 -> out

Matmul inputs are bf16 (weights pre-cast on host), PSUM accumulation f32,
element-wise math f32. PSUM->SBUF moves go to ScalarE where DVE is the
phase bottleneck.
"""

import numpy as np
import ml_dtypes

import concourse.bass as bass
import concourse.mybir as mybir
import concourse.tile as tile
from concourse import bacc
from concourse import bass_utils
from concourse.masks import make_identity

BF = mybir.dt.bfloat16
F32 = mybir.dt.float32
AF = mybir.ActivationFunctionType
OP = mybir.AluOpType

B, S, D, H, Dh, Fd = 8, 512, 768, 12, 64, 3072
NCORES = 8
MASK_NEG = -30.0  # effectively -inf after exp given |scores+gauss| <~ 8
EPS = 1e-12

KD = D // 128      # 6  K-tiles over D
MS = S // 128      # 4  chunks over sequence
KF = Fd // 128     # 24 K-tiles over Fd
NT = 2             # N-tiles over D for natural-layout outputs (2 x 384)
ND = D // NT       # 384


def _trace(nc, io):
    with tile.TileContext(nc) as tc:
        _trace_body(nc, tc, io)


def _trace_body(nc, tc, io):
    from contextlib import ExitStack

    with ExitStack() as ctx:
        glob = ctx.enter_context(tc.tile_pool(name="glob", bufs=1))
        psum = ctx.enter_context(tc.tile_pool(name="psum", bufs=1, space="PSUM"))

        # ---- constants / small inputs ----
        ident_bf = glob.tile([128, 128], BF, tag="ident")
        make_identity(nc, ident_bf)
        ones_bf = glob.tile([1, 512], BF, tag="ones_bf")
        nc.vector.memset(ones_bf, 1.0)
        ones64_f = glob.tile([1, 64], F32, tag="ones64")
        nc.vector.memset(ones64_f, 1.0)
        eps_t = glob.tile([128, 1], F32, tag="eps")
        nc.vector.memset(eps_t, EPS)

        bq8_c = glob.tile([128, KD], F32, tag="bq8")
        nc.gpsimd.dma_start(out=bq8_c, in_=io["bq8"].rearrange("(c p) -> p c", p=128))
        bk_c = glob.tile([128, KD], F32, tag="bk")
        nc.gpsimd.dma_start(out=bk_c, in_=io["bk"].rearrange("(c p) -> p c", p=128))
        b1_c = glob.tile([128, KF], F32, tag="b1")
        nc.gpsimd.dma_start(out=b1_c, in_=io["b1"].rearrange("(c p) -> p c", p=128))

        bv_r = glob.tile([1, D], BF, tag="bv")
        nc.gpsimd.dma_start(out=bv_r, in_=io["bv_bf"].rearrange("(a n) -> a n", a=1))
        bproj_r = glob.tile([1, D], BF, tag="bproj")
        nc.gpsimd.dma_start(out=bproj_r, in_=io["bproj_bf"].rearrange("(a n) -> a n", a=1))
        b2_r = glob.tile([1, D], BF, tag="b2")
        nc.gpsimd.dma_start(out=b2_r, in_=io["b2_bf"].rearrange("(a n) -> a n", a=1))

        def bcast128(ap):
            return bass.AP(tensor=ap.tensor, offset=ap.offset, ap=[[0, 128]] + list(ap.ap))

        g1b = glob.tile([128, D], F32, tag="g1b")
        nc.gpsimd.dma_start(out=g1b, in_=bcast128(io["gamma1"]))
        be1b = glob.tile([128, D], F32, tag="be1b")
        nc.gpsimd.dma_start(out=be1b, in_=bcast128(io["beta1"]))
        g2b = glob.tile([128, D], F32, tag="g2b")
        nc.gpsimd.dma_start(out=g2b, in_=bcast128(io["gamma2"]))
        be2b = glob.tile([128, D], F32, tag="be2b")
        nc.gpsimd.dma_start(out=be2b, in_=bcast128(io["beta2"]))

        h1_sb = glob.tile([128, MS, D], F32, tag="h1")
        h1T_sb = glob.tile([128, KD, S], BF, tag="h1T")
        ff1T_sb = glob.tile([128, KF, S], BF, tag="ff1T")

        # ================= attention scope =================
        with tc.tile_pool(name="attn", bufs=1) as attnp:
            # weight DMAs first: the first matmuls need wq/wk + xT.
            wq_sb = attnp.tile([128, KD, D], BF, tag="wq")
            nc.sync.dma_start(out=wq_sb, in_=io["wq_bf"].rearrange("(c p) n -> p c n", p=128))
            wk_sb = attnp.tile([128, KD, D], BF, tag="wk")
            nc.sync.dma_start(out=wk_sb, in_=io["wk_bf"].rearrange("(c p) n -> p c n", p=128))
            xT_sb = attnp.tile([128, KD, S], BF, tag="xT")
            nc.sync.dma_start(out=xT_sb, in_=io["xT_bf"].rearrange("(c p) s -> p c s", p=128))

            qT_sb = attnp.tile([128, KD, S], BF, tag="qT")
            kT_sb = attnp.tile([128, KD, S], BF, tag="kT")
            v_sb = attnp.tile([128, MS, H, Dh + 1], BF, tag="v")
            nc.vector.memset(v_sb[:, :, :, Dh : Dh + 1], 1.0)
            hT_sb = attnp.tile([128, KD, S], BF, tag="hT")

            # --- qT / kT projections (T layout); bias (+0.125 scale for q) on ACT ---
            for w_sb, dst, bias_c, scale in (
                (wq_sb, qT_sb, bq8_c, 0.125),
                (wk_sb, kT_sb, bk_c, 1.0),
            ):
                for m in range(KD):
                    ps = psum.tile([128, 512], F32, tag="acc", bufs=2, name="ps_qk")
                    for k in range(KD):
                        nc.tensor.matmul(
                            ps, w_sb[:, k, 128 * m : 128 * (m + 1)], xT_sb[:, k, :],
                            start=(k == 0), stop=(k == KD - 1),
                        )
                    nc.scalar.activation(
                        out=dst[:, m, :], in_=ps, func=AF.Identity,
                        bias=bias_c[:, m : m + 1], scale=scale,
                    )

            # --- v projection (natural layout, bias via K-augmentation) ---
            wv_sb = attnp.tile([128, KD, D], BF, tag="wv")
            nc.sync.dma_start(out=wv_sb, in_=io["wv_bf"].rearrange("(c p) n -> p c n", p=128))
            gauss_sb = attnp.tile([128, MS, S], BF, tag="gauss")
            nc.sync.dma_start(out=gauss_sb, in_=io["gexp"].rearrange("(c p) s -> p c s", p=128))
            x_sb = glob.tile([128, MS, D], F32, tag="x")
            nc.sync.dma_start(out=x_sb, in_=io["x"].rearrange("(c p) n -> p c n", p=128))

            for c in range(MS):
                for n in range(NT):
                    ps = psum.tile([128, ND], F32, tag="acc", bufs=2, name="ps_v")
                    for k in range(KD):
                        nc.tensor.matmul(
                            ps, xT_sb[:, k, 128 * c : 128 * (c + 1)],
                            wv_sb[:, k, ND * n : ND * (n + 1)],
                            start=(k == 0), stop=False,
                        )
                    nc.tensor.matmul(
                        ps, ones_bf[:, 0:128], bv_r[:, ND * n : ND * (n + 1)],
                        start=False, stop=True,
                    )
                    nc.scalar.copy(
                        out=v_sb[:, c, 6 * n : 6 * (n + 1), 0:Dh],
                        in_=ps.rearrange("p (h d) -> p h d", d=Dh),
                    )

            # --- per-head attention ---
            for h in range(H):
                th, off = h // 2, (h % 2) * 64
                qh = qT_sb[off : off + 64, th, :]
                kh = kT_sb[off : off + 64, th, :]
                exp_tiles = []
                for c in range(MS):
                    ps_sc = psum.tile([128, 512], F32, tag="sc", bufs=3, name="ps_sc")
                    nc.tensor.matmul(
                        ps_sc, kh[:, 128 * c : 128 * (c + 1)], qh, start=True, stop=True
                    )
                    exraw = attnp.tile([128, 512], BF, tag="exraw", bufs=6, name="exraw")
                    nc.scalar.activation(out=exraw, in_=ps_sc, func=AF.Exp)
                    ex = attnp.tile([128, 512], BF, tag="exp", bufs=6, name="ex")
                    nc.vector.tensor_tensor(
                        out=ex, in0=exraw, in1=gauss_sb[:, c, :], op=OP.mult
                    )
                    exp_tiles.append(ex)
                ps_h = psum.tile([Dh + 1, 512], F32, tag="hT", bufs=2, name="ps_h")
                for c in range(MS):
                    nc.tensor.matmul(
                        ps_h, v_sb[:, c, h, :], exp_tiles[c],
                        start=(c == 0), stop=(c == MS - 1),
                    )
                srow = attnp.tile([1, 512], F32, tag="srow", bufs=3, name="srow")
                nc.scalar.copy(out=srow, in_=ps_h[Dh : Dh + 1, :])
                rec = attnp.tile([1, 512], F32, tag="rec", bufs=3, name="rec")
                nc.vector.reciprocal_approx_fast(out=rec, in_=srow)
                ps_b = psum.tile([64, 512], F32, tag="bc", bufs=1, name="ps_b")
                nc.tensor.matmul(ps_b, ones64_f, rec, start=True, stop=True)
                bca = attnp.tile([64, 512], F32, tag="bca", bufs=3, name="bca")
                nc.vector.tensor_copy(bca, ps_b)
                nc.vector.tensor_tensor(
                    out=hT_sb[off : off + 64, th, :], in0=ps_h[0:64, :], in1=bca, op=OP.mult
                )

            # --- proj + residual + LN1 (+ h1 transpose) ---
            wp_sb = attnp.tile([128, KD, D], BF, tag="wp")
            nc.sync.dma_start(out=wp_sb, in_=io["wproj_bf"].rearrange("(c p) n -> p c n", p=128))
            for m in range(MS):
                pss = []
                for n in range(NT):
                    ps = psum.tile([128, ND], F32, tag="acc", bufs=2, name="ps_pr")
                    for k in range(KD):
                        nc.tensor.matmul(
                            ps, hT_sb[:, k, 128 * m : 128 * (m + 1)],
                            wp_sb[:, k, ND * n : ND * (n + 1)],
                            start=(k == 0), stop=False,
                        )
                    nc.tensor.matmul(
                        ps, ones_bf[:, 0:128], bproj_r[:, ND * n : ND * (n + 1)],
                        start=False, stop=True,
                    )
                    pss.append(ps)
                row = glob.tile([128, D], F32, tag="rowtmp", bufs=3, name="row")
                for n in range(NT):
                    nc.vector.tensor_tensor(
                        out=row[:, ND * n : ND * (n + 1)], in0=pss[n],
                        in1=x_sb[:, m, ND * n : ND * (n + 1)], op=OP.add,
                    )
                _layernorm(nc, glob, row, g1b, be1b, eps_t, h1_sb[:, m, :])
                h1bf = glob.tile([128, D], BF, tag="h1bf", bufs=2, name="h1bf")
                nc.vector.tensor_copy(h1bf, h1_sb[:, m, :])
                for f in range(KD):
                    ps_t = psum.tile([128, 128], BF, tag="acc", bufs=2, name="ps_t")
                    nc.tensor.transpose(ps_t, h1bf[:, 128 * f : 128 * (f + 1)], ident_bf)
                    nc.scalar.copy(out=h1T_sb[:, f, 128 * m : 128 * (m + 1)], in_=ps_t)

        # ================= FFN scope =================
        with tc.tile_pool(name="ffn", bufs=1) as ffnp:
            w1_sb = ffnp.tile([128, KD, Fd], BF, tag="w1")
            nc.sync.dma_start(out=w1_sb, in_=io["w1_bf"].rearrange("(c p) n -> p c n", p=128))
            for fm in range(KF):
                ps = psum.tile([128, 512], F32, tag="acc", bufs=2, name="ps_f1")
                for k in range(KD):
                    nc.tensor.matmul(
                        ps, w1_sb[:, k, 128 * fm : 128 * (fm + 1)], h1T_sb[:, k, :],
                        start=(k == 0), stop=(k == KD - 1),
                    )
                nc.scalar.activation(
                    out=ff1T_sb[:, fm, :], in_=ps, func=AF.Gelu,
                    bias=b1_c[:, fm : fm + 1], scale=1.0,
                )

            w2_sb = ffnp.tile([128, KF, D], BF, tag="w2")
            nc.sync.dma_start(out=w2_sb, in_=io["w2_bf"].rearrange("(c p) n -> p c n", p=128))
            for m in range(MS):
                pss = []
                for n in range(NT):
                    ps = psum.tile([128, ND], F32, tag="acc", bufs=2, name="ps_f2")
                    for k in range(KF):
                        nc.tensor.matmul(
                            ps, ff1T_sb[:, k, 128 * m : 128 * (m + 1)],
                            w2_sb[:, k, ND * n : ND * (n + 1)],
                            start=(k == 0), stop=False,
                        )
                    nc.tensor.matmul(
                        ps, ones_bf[:, 0:128], b2_r[:, ND * n : ND * (n + 1)],
                        start=False, stop=True,
                    )
                    pss.append(ps)
                row = glob.tile([128, D], F32, tag="rowtmp", bufs=3, name="row2")
                for n in range(NT):
                    nc.vector.tensor_tensor(
                        out=row[:, ND * n : ND * (n + 1)], in0=pss[n],
                        in1=h1_sb[:, m, ND * n : ND * (n + 1)], op=OP.add,
                    )
                outrow = glob.tile([128, D], F32, tag="outrow", bufs=2, name="outrow")
                _layernorm(nc, glob, row, g2b, be2b, eps_t, outrow)
                nc.sync.dma_start(
                    out=io["out"][128 * m : 128 * (m + 1), :], in_=outrow
                )


def _layernorm(nc, pool, row, gamma_b, beta_b, eps_t, out_ap):
    st = pool.tile([128, 3, 6], F32, tag="st", bufs=4, name="st")
    for g in range(3):
        nc.vector.bn_stats(out=st[:, g, :], in_=row[:, 256 * g : 256 * (g + 1)])
    mv = pool.tile([128, 2], F32, tag="mv", bufs=4, name="mv")
    nc.vector.bn_aggr(out=mv, in_=st)
    sd = pool.tile([128, 1], F32, tag="sd", bufs=4, name="sd")
    nc.scalar.activation(out=sd, in_=mv[:, 1:2], func=AF.Sqrt, bias=eps_t, scale=1.0)
    rs = pool.tile([128, 1], F32, tag="rs", bufs=4, name="rs")
    nc.vector.reciprocal(rs, sd)
    # in-place: row = (row - mean) * gamma ; out = row * rstd + beta
    nc.vector.scalar_tensor_tensor(
        out=row, in0=row, scalar=mv[:, 0:1], in1=gamma_b, op0=OP.subtract, op1=OP.mult
    )
    nc.vector.scalar_tensor_tensor(
        out=out_ap, in0=row, scalar=rs, in1=beta_b, op0=OP.mult, op1=OP.add
    )


_SPECS = [
    # (name, shape, dtype)
    ("x", [S, D], F32),
    ("xT_bf", [D, S], BF),
    ("gexp", [S, S], BF),
    ("wq_bf", [D, D], BF),
    ("wk_bf", [D, D], BF),
    ("wv_bf", [D, D], BF),
    ("wproj_bf", [D, D], BF),
    ("w1_bf", [D, Fd], BF),
    ("w2_bf", [Fd, D], BF),
    ("bq8", [D], F32),
    ("bk", [D], F32),
    ("bv_bf", [D], BF),
    ("bproj_bf", [D], BF),
    ("b1", [Fd], F32),
    ("b2_bf", [D], BF),
    ("gamma1", [D], F32),
    ("beta1", [D], F32),
    ("gamma2", [D], F32),
    ("beta2", [D], F32),
]

_BUILT = {}


def _build():
    if "nc" in _BUILT:
        return _BUILT["nc"]
    nc = bacc.Bacc("TRN2", target_bir_lowering=False, debug=False,
                   enable_asserts=False, num_devices=NCORES)
    io = {}
    for name, shape, dt in _SPECS:
        io[name] = nc.dram_tensor(name, shape, dt, kind="ExternalInput").ap()
    io["out"] = nc.dram_tensor("out", [S, D], F32, kind="ExternalOutput").ap()
    _trace(nc, io)
    nc.compile()
    _BUILT["nc"] = nc
    return nc


def _host_prep(inputs):
    bf = ml_dtypes.bfloat16
    f32 = np.float32
    x = np.asarray(inputs["x"], f32)
    mask = np.asarray(inputs["mask"])

    idx = np.arange(S, dtype=np.float64)
    dd = idx[None, :] - idx[:, None]
    sc = -0.5 * dd * dd
    sc -= sc.max(axis=-1, keepdims=True)
    e = np.exp(sc)
    gauss = (e / e.sum(axis=-1, keepdims=True)).astype(f32)  # [i=s, j=t]
    gaussT = np.ascontiguousarray(gauss.T)

    shared = {
        "wq_bf": np.asarray(inputs["wq"], f32).astype(bf),
        "wk_bf": np.asarray(inputs["wk"], f32).astype(bf),
        "wv_bf": np.asarray(inputs["wv"], f32).astype(bf),
        "wproj_bf": np.asarray(inputs["w_proj"], f32).astype(bf),
        "w1_bf": np.asarray(inputs["w1"], f32).astype(bf),
        "w2_bf": np.asarray(inputs["w2"], f32).astype(bf),
        "bq8": np.asarray(inputs["bq"], f32) * np.float32(0.125),
        "bk": np.asarray(inputs["bk"], f32),
        "bv_bf": np.asarray(inputs["bv"], f32).astype(bf),
        "bproj_bf": np.asarray(inputs["b_proj"], f32).astype(bf),
        "b1": np.asarray(inputs["b1"], f32),
        "b2_bf": np.asarray(inputs["b2"], f32).astype(bf),
        "gamma1": np.asarray(inputs["gamma1"], f32),
        "beta1": np.asarray(inputs["beta1"], f32),
        "gamma2": np.asarray(inputs["gamma2"], f32),
        "beta2": np.asarray(inputs["beta2"], f32),
    }
    in_maps = []
    for b in range(NCORES):
        m = dict(shared)
        m["x"] = np.ascontiguousarray(x[b])
        m["xT_bf"] = np.ascontiguousarray(x[b].T).astype(bf)
        gm = gaussT + (MASK_NEG * (1.0 - mask[b].astype(f32)))[:, None]
        m["gexp"] = np.exp(gm).astype(bf)
        in_maps.append(m)
    return in_maps


def _run(inputs, trace=False, trace_cores=None):
    nc = _build()
    in_maps = _host_prep(inputs)
    res = bass_utils.run_bass_kernel_spmd(
        nc, in_maps, core_ids=list(range(NCORES)), trace=trace,
        trace_cores=trace_cores,
    )
    out = np.stack([np.asarray(res.results[b]["out"]) for b in range(NCORES)])
    return out.astype(np.float32), res


def kernel(**inputs):
    return _run(inputs)[0]


# revision 10
# speedup vs baseline: 1.2192x; 1.0935x over previous
"""Trainium2 Bass kernel for a dense transformer block (B=8, S=512, D=768, H=12, Fd=3072).

Sharding: pure data-parallel over batch — one batch element per NeuronCore,
weights replicated, no collectives.

Math layout trick: activations are kept feature-major ("T layout", [feat, seq])
through the attention pipeline so the TensorEngine (which contracts over the
partition dim) never needs an on-device transpose of the attention matrix:

  xT [768,512] (host-transposed)
  qT = wq.T @ xT, kT = wk.T @ xT          (T layout, per-partition bias via ACT)
  v  = xT.T @ wv (natural [t,d] layout, bias via K-augmentation)
  scoresT[t,s] = kT_h[:,tchunk].T @ qT_h
  expT = exp(scoresT + gmaskT[t,s])       (gmask = gauss bias + mask penalty,
                                           host-precomputed; no max-subtraction
                                           needed since scores are O(1))
  hT_aug[d+1, s] = [v_h | 1].T @ expT     (ones-column gives the softmax sums)
  hT = hT_aug[:64] * broadcast(1/sum)     (recip_approx_fast + rank-1 matmul bcast)
  proj = hT_all.T @ wproj (+ bias row)    -> natural [s, 768]; residual + LN1
  h1T  = PE-transpose(h1)                 (24 128x128 transposes)
  ff1T = w1.T @ h1T, gelu fused on PSUM->SBUF copy with per-partition b1
  ff2  = ff1T.T @ w2 (+ bias row)         -> natural; residual + LN2 -> out

Matmul inputs are bf16 (weights pre-cast on host), PSUM accumulation f32,
element-wise math f32. PSUM->SBUF moves go to ScalarE where DVE is the
phase bottleneck.
"""

import numpy as np
import ml_dtypes

import concourse.bass as bass
import concourse.mybir as mybir
import concourse.tile as tile
from concourse import bacc
from concourse import bass_utils
from concourse.masks import make_identity

BF = mybir.dt.bfloat16
F32 = mybir.dt.float32
AF = mybir.ActivationFunctionType
OP = mybir.AluOpType

B, S, D, H, Dh, Fd = 8, 512, 768, 12, 64, 3072
NCORES = 8
MASK_NEG = -30.0  # effectively -inf after exp given |scores+gauss| <~ 8
EPS = 1e-12

KD = D // 128      # 6  K-tiles over D
MS = S // 128      # 4  chunks over sequence
KF = Fd // 128     # 24 K-tiles over Fd
NT = 2             # N-tiles over D for natural-layout outputs (2 x 384)
ND = D // NT       # 384


def _trace(nc, io):
    with tile.TileContext(nc) as tc:
        _trace_body(nc, tc, io)


def _trace_body(nc, tc, io):
    from contextlib import ExitStack

    with ExitStack() as ctx:
        glob = ctx.enter_context(tc.tile_pool(name="glob", bufs=1))
        psum = ctx.enter_context(tc.tile_pool(name="psum", bufs=1, space="PSUM"))

        # ---- constants / small inputs ----
        ident_bf = glob.tile([128, 128], BF, tag="ident")
        make_identity(nc, ident_bf)
        ones_bf = glob.tile([1, 512], BF, tag="ones_bf")
        nc.vector.memset(ones_bf, 1.0)
        ones64_f = glob.tile([1, 64], F32, tag="ones64")
        nc.vector.memset(ones64_f, 1.0)
        eps_t = glob.tile([128, 1], F32, tag="eps")
        nc.vector.memset(eps_t, EPS)

        biasf_sb = glob.tile([128, 2 * KD + KF], F32, tag="biasf")
        nc.scalar.dma_start(out=biasf_sb, in_=io["bias_f"])
        bq8_c = biasf_sb[:, 0:KD]
        bk_c = biasf_sb[:, KD : 2 * KD]
        b1_c = biasf_sb[:, 2 * KD : 2 * KD + KF]

        biasb_sb = glob.tile([1, 3 * D], BF, tag="biasb")
        nc.scalar.dma_start(out=biasb_sb, in_=io["bias_b"])
        bv_r = biasb_sb[:, 0:D]
        bproj_r = biasb_sb[:, D : 2 * D]
        b2_r = biasb_sb[:, 2 * D : 3 * D]

        gbt = glob.tile([128, 4, D], F32, tag="gbt")
        nc.gpsimd.dma_start(
            out=gbt,
            in_=bass.AP(tensor=io["gb"].tensor, offset=io["gb"].offset,
                        ap=[[0, 128]] + list(io["gb"].ap)),
        )
        g1b, be1b, g2b, be2b = gbt[:, 0, :], gbt[:, 1, :], gbt[:, 2, :], gbt[:, 3, :]

        h1_sb = glob.tile([128, MS, D], F32, tag="h1")
        h1T_sb = glob.tile([128, KD, S], BF, tag="h1T")
        ff1T_sb = glob.tile([128, KF, S], BF, tag="ff1T")

        # ================= attention scope =================
        with tc.tile_pool(name="attn", bufs=1) as attnp:
            # weight DMAs first: the first matmuls need wq/wk + xT.
            wq_sb = attnp.tile([128, KD, D], BF, tag="wq")
            nc.sync.dma_start(out=wq_sb.rearrange("p c n -> p (c n)"), in_=io["wq_bf"])
            wk_sb = attnp.tile([128, KD, D], BF, tag="wk")
            nc.sync.dma_start(out=wk_sb.rearrange("p c n -> p (c n)"), in_=io["wk_bf"])
            xT_sb = attnp.tile([128, KD, S], BF, tag="xT")
            nc.scalar.dma_start(out=xT_sb.rearrange("p c s -> p (c s)"), in_=io["xT_bf"])

            qT_sb = attnp.tile([128, KD, S], BF, tag="qT")
            kT_sb = attnp.tile([128, KD, S], BF, tag="kT")
            v_sb = attnp.tile([128, MS, H, Dh + 1], BF, tag="v")
            nc.vector.memset(v_sb[:, :, :, Dh : Dh + 1], 1.0)
            hT_sb = attnp.tile([128, KD, S], BF, tag="hT")

            # --- qT / kT projections (T layout); bias (+0.125 scale for q) on ACT ---
            for w_sb, dst, bias_c, scale in (
                (wq_sb, qT_sb, bq8_c, 0.125),
                (wk_sb, kT_sb, bk_c, 1.0),
            ):
                for m in range(KD):
                    ps = psum.tile([128, 512], F32, tag="acc", bufs=2, name="ps_qk")
                    for k in range(KD):
                        nc.tensor.matmul(
                            ps, w_sb[:, k, 128 * m : 128 * (m + 1)], xT_sb[:, k, :],
                            start=(k == 0), stop=(k == KD - 1),
                        )
                    nc.scalar.activation(
                        out=dst[:, m, :], in_=ps, func=AF.Identity,
                        bias=bias_c[:, m : m + 1], scale=scale,
                    )

            # --- v projection (natural layout, bias via K-augmentation) ---
            wv_sb = attnp.tile([128, KD, D], BF, tag="wv")
            nc.sync.dma_start(out=wv_sb.rearrange("p c n -> p (c n)"), in_=io["wv_bf"])
            gauss_sb = attnp.tile([128, MS, S], BF, tag="gauss")
            nc.scalar.dma_start(out=gauss_sb.rearrange("p c s -> p (c s)"), in_=io["gexp"])
            x_sb = glob.tile([128, MS, D], F32, tag="x")
            nc.scalar.dma_start(out=x_sb.rearrange("p c n -> p (c n)"), in_=io["x"])

            for c in range(MS):
                for n in range(NT):
                    ps = psum.tile([128, ND], F32, tag="acc", bufs=2, name="ps_v")
                    for k in range(KD):
                        nc.tensor.matmul(
                            ps, xT_sb[:, k, 128 * c : 128 * (c + 1)],
                            wv_sb[:, k, ND * n : ND * (n + 1)],
                            start=(k == 0), stop=False,
                        )
                    nc.tensor.matmul(
                        ps, ones_bf[:, 0:128], bv_r[:, ND * n : ND * (n + 1)],
                        start=False, stop=True,
                    )
                    nc.scalar.copy(
                        out=v_sb[:, c, 6 * n : 6 * (n + 1), 0:Dh],
                        in_=ps.rearrange("p (h d) -> p h d", d=Dh),
                    )

            # --- per-head attention ---
            for h in range(H):
                th, off = h // 2, (h % 2) * 64
                qh = qT_sb[off : off + 64, th, :]
                kh = kT_sb[off : off + 64, th, :]
                exp_tiles = []
                for half in range(2):
                    ps_sc = psum.tile([128, 2, 512], F32, tag="sc2", bufs=2, name="ps_sc")
                    for j in range(2):
                        c = 2 * half + j
                        nc.tensor.matmul(
                            ps_sc[:, j, :], kh[:, 128 * c : 128 * (c + 1)], qh,
                            start=True, stop=True,
                        )
                    exraw = attnp.tile([128, 2, 512], BF, tag="exraw", bufs=4, name="exraw")
                    nc.scalar.activation(out=exraw, in_=ps_sc, func=AF.Exp)
                    ex = attnp.tile([128, 2, 512], BF, tag="exp", bufs=4, name="ex")
                    nc.vector.tensor_tensor(
                        out=ex, in0=exraw,
                        in1=gauss_sb[:, 2 * half : 2 * half + 2, :], op=OP.mult,
                    )
                    exp_tiles.append(ex)
                ps_h = psum.tile([Dh + 1, 512], F32, tag="hT", bufs=2, name="ps_h")
                for c in range(MS):
                    nc.tensor.matmul(
                        ps_h, v_sb[:, c, h, :], exp_tiles[c // 2][:, c % 2, :],
                        start=(c == 0), stop=(c == MS - 1),
                    )
                srow = attnp.tile([1, 512], F32, tag="srow", bufs=3, name="srow")
                nc.scalar.copy(out=srow, in_=ps_h[Dh : Dh + 1, :])
                rec = attnp.tile([1, 512], F32, tag="rec", bufs=3, name="rec")
                nc.vector.reciprocal_approx_fast(out=rec, in_=srow)
                ps_b = psum.tile([64, 512], F32, tag="acc", bufs=2, name="ps_b")
                nc.tensor.matmul(ps_b, ones64_f, rec, start=True, stop=True)
                bca = attnp.tile([64, 512], F32, tag="bca", bufs=3, name="bca")
                nc.vector.tensor_copy(bca, ps_b)
                nc.vector.tensor_tensor(
                    out=hT_sb[off : off + 64, th, :], in0=ps_h[0:64, :], in1=bca, op=OP.mult
                )

            # --- proj + residual + LN1 (+ h1 transpose) ---
            wp_sb = attnp.tile([128, KD, D], BF, tag="wp")
            nc.sync.dma_start(out=wp_sb.rearrange("p c n -> p (c n)"), in_=io["wproj_bf"])
            for m in range(MS):
                pss = []
                for n in range(NT):
                    ps = psum.tile([128, ND], F32, tag="acc", bufs=2, name="ps_pr")
                    for k in range(KD):
                        nc.tensor.matmul(
                            ps, hT_sb[:, k, 128 * m : 128 * (m + 1)],
                            wp_sb[:, k, ND * n : ND * (n + 1)],
                            start=(k == 0), stop=False,
                        )
                    nc.tensor.matmul(
                        ps, ones_bf[:, 0:128], bproj_r[:, ND * n : ND * (n + 1)],
                        start=False, stop=True,
                    )
                    pss.append(ps)
                row = glob.tile([128, D], F32, tag="rowtmp", bufs=3, name="row")
                for n in range(NT):
                    nc.vector.tensor_tensor(
                        out=row[:, ND * n : ND * (n + 1)], in0=pss[n],
                        in1=x_sb[:, m, ND * n : ND * (n + 1)], op=OP.add,
                    )
                _layernorm(nc, glob, row, g1b, be1b, eps_t, h1_sb[:, m, :])
                h1bf = glob.tile([128, D], BF, tag="h1bf", bufs=2, name="h1bf")
                nc.vector.tensor_copy(h1bf, h1_sb[:, m, :])
                for f in range(KD):
                    ps_t = psum.tile([128, 128], BF, tag="sc2", bufs=2, name="ps_t")
                    nc.tensor.transpose(ps_t, h1bf[:, 128 * f : 128 * (f + 1)], ident_bf)
                    nc.scalar.copy(out=h1T_sb[:, f, 128 * m : 128 * (m + 1)], in_=ps_t)

        # ================= FFN scope =================
        with tc.tile_pool(name="ffn", bufs=1) as ffnp:
            w1_sb = ffnp.tile([128, KD, Fd], BF, tag="w1")
            nc.sync.dma_start(out=w1_sb.rearrange("p c n -> p (c n)"), in_=io["w1_bf"])
            for fm in range(KF):
                ps = psum.tile([128, 512], F32, tag="acc", bufs=2, name="ps_f1")
                for k in range(KD):
                    nc.tensor.matmul(
                        ps, w1_sb[:, k, 128 * fm : 128 * (fm + 1)], h1T_sb[:, k, :],
                        start=(k == 0), stop=(k == KD - 1),
                    )
                nc.scalar.activation(
                    out=ff1T_sb[:, fm, :], in_=ps, func=AF.Gelu,
                    bias=b1_c[:, fm : fm + 1], scale=1.0,
                )

            w2_sb = ffnp.tile([128, KF, D], BF, tag="w2")
            nc.sync.dma_start(out=w2_sb.rearrange("p c n -> p (c n)"), in_=io["w2_bf"])
            for m in range(MS):
                pss = []
                for n in range(NT):
                    ps = psum.tile([128, ND], F32, tag="acc", bufs=2, name="ps_f2")
                    for k in range(KF):
                        nc.tensor.matmul(
                            ps, ff1T_sb[:, k, 128 * m : 128 * (m + 1)],
                            w2_sb[:, k, ND * n : ND * (n + 1)],
                            start=(k == 0), stop=False,
                        )
                    nc.tensor.matmul(
                        ps, ones_bf[:, 0:128], b2_r[:, ND * n : ND * (n + 1)],
                        start=False, stop=True,
                    )
                    pss.append(ps)
                row = glob.tile([128, D], F32, tag="rowtmp", bufs=3, name="row2")
                for n in range(NT):
                    nc.vector.tensor_tensor(
                        out=row[:, ND * n : ND * (n + 1)], in0=pss[n],
                        in1=h1_sb[:, m, ND * n : ND * (n + 1)], op=OP.add,
                    )
                outrow = glob.tile([128, D], F32, tag="outrow", bufs=2, name="outrow")
                _layernorm(nc, glob, row, g2b, be2b, eps_t, outrow)
                nc.sync.dma_start(
                    out=io["out"][128 * m : 128 * (m + 1), :], in_=outrow
                )


def _layernorm(nc, pool, row, gamma_b, beta_b, eps_t, out_ap):
    st = pool.tile([128, 3, 6], F32, tag="st", bufs=4, name="st")
    for g in range(3):
        nc.vector.bn_stats(out=st[:, g, :], in_=row[:, 256 * g : 256 * (g + 1)])
    mv = pool.tile([128, 2], F32, tag="mv", bufs=4, name="mv")
    nc.vector.bn_aggr(out=mv, in_=st)
    sd = pool.tile([128, 1], F32, tag="sd", bufs=4, name="sd")
    nc.scalar.activation(out=sd, in_=mv[:, 1:2], func=AF.Sqrt, bias=eps_t, scale=1.0)
    rs = pool.tile([128, 1], F32, tag="rs", bufs=4, name="rs")
    nc.vector.reciprocal(rs, sd)
    # in-place: row = (row - mean) * gamma ; out = row * rstd + beta
    nc.vector.scalar_tensor_tensor(
        out=row, in0=row, scalar=mv[:, 0:1], in1=gamma_b, op0=OP.subtract, op1=OP.mult
    )
    nc.vector.scalar_tensor_tensor(
        out=out_ap, in0=row, scalar=rs, in1=beta_b, op0=OP.mult, op1=OP.add
    )


_SPECS = [
    # (name, shape, dtype) — big tensors pre-permuted on host to SBUF layout
    ("x", [128, MS * D], F32),
    ("xT_bf", [128, KD * S], BF),
    ("gexp", [128, MS * S], BF),
    ("wq_bf", [128, KD * D], BF),
    ("wk_bf", [128, KD * D], BF),
    ("wv_bf", [128, KD * D], BF),
    ("wproj_bf", [128, KD * D], BF),
    ("w1_bf", [128, KD * Fd], BF),
    ("w2_bf", [128, KF * D], BF),
    ("bias_f", [128, 2 * KD + KF], F32),   # bq8 | bk | b1, per-partition cols
    ("bias_b", [1, 3 * D], BF),            # bv | bproj | b2 rows
    ("gb", [4 * D], F32),                  # gamma1|beta1|gamma2|beta2 (bcast)
]

_BUILT = {}


def _build():
    if "nc" in _BUILT:
        return _BUILT["nc"]
    nc = bacc.Bacc("TRN2", target_bir_lowering=False, debug=False,
                   enable_asserts=False, num_devices=NCORES)
    io = {}
    for name, shape, dt in _SPECS:
        io[name] = nc.dram_tensor(name, shape, dt, kind="ExternalInput").ap()
    io["out"] = nc.dram_tensor("out", [S, D], F32, kind="ExternalOutput").ap()
    _trace(nc, io)
    nc.compile()
    _BUILT["nc"] = nc
    return nc


def _host_prep(inputs):
    bf = ml_dtypes.bfloat16
    f32 = np.float32
    x = np.asarray(inputs["x"], f32)
    mask = np.asarray(inputs["mask"])

    idx = np.arange(S, dtype=np.float64)
    dd = idx[None, :] - idx[:, None]
    sc = -0.5 * dd * dd
    sc -= sc.max(axis=-1, keepdims=True)
    e = np.exp(sc)
    gauss = (e / e.sum(axis=-1, keepdims=True)).astype(f32)  # [i=s, j=t]
    gaussT = np.ascontiguousarray(gauss.T)

    def sbl(a, p=128):  # [C*p, N] -> [p, C*N] (SBUF layout)
        cN = a.shape[0] // p
        return np.ascontiguousarray(
            a.reshape(cN, p, a.shape[1]).transpose(1, 0, 2).reshape(p, -1)
        )

    def pcols(a, p=128):  # [C*p] -> [p, C] per-partition columns
        return np.ascontiguousarray(a.reshape(-1, p).T)

    bias_f = np.concatenate(
        [
            pcols(np.asarray(inputs["bq"], f32) * np.float32(0.125)),
            pcols(np.asarray(inputs["bk"], f32)),
            pcols(np.asarray(inputs["b1"], f32)),
        ],
        axis=1,
    )
    bias_b = np.concatenate(
        [
            np.asarray(inputs["bv"], f32),
            np.asarray(inputs["b_proj"], f32),
            np.asarray(inputs["b2"], f32),
        ]
    ).astype(bf)[None, :]
    gb = np.concatenate(
        [
            np.asarray(inputs["gamma1"], f32),
            np.asarray(inputs["beta1"], f32),
            np.asarray(inputs["gamma2"], f32),
            np.asarray(inputs["beta2"], f32),
        ]
    )
    shared = {
        "wq_bf": sbl(np.asarray(inputs["wq"], f32).astype(bf)),
        "wk_bf": sbl(np.asarray(inputs["wk"], f32).astype(bf)),
        "wv_bf": sbl(np.asarray(inputs["wv"], f32).astype(bf)),
        "wproj_bf": sbl(np.asarray(inputs["w_proj"], f32).astype(bf)),
        "w1_bf": sbl(np.asarray(inputs["w1"], f32).astype(bf)),
        "w2_bf": sbl(np.asarray(inputs["w2"], f32).astype(bf)),
        "bias_f": bias_f,
        "bias_b": bias_b,
        "gb": gb,
    }
    in_maps = []
    for b in range(NCORES):
        m = dict(shared)
        m["x"] = sbl(np.ascontiguousarray(x[b]))
        m["xT_bf"] = sbl(np.ascontiguousarray(x[b].T).astype(bf))
        gm = gaussT + (MASK_NEG * (1.0 - mask[b].astype(f32)))[:, None]
        m["gexp"] = sbl(np.exp(gm).astype(bf))
        in_maps.append(m)
    return in_maps


def _run(inputs, trace=False, trace_cores=None):
    nc = _build()
    in_maps = _host_prep(inputs)
    res = bass_utils.run_bass_kernel_spmd(
        nc, in_maps, core_ids=list(range(NCORES)), trace=trace,
        trace_cores=trace_cores,
    )
    out = np.stack([np.asarray(res.results[b]["out"]) for b in range(NCORES)])
    return out.astype(np.float32), res


def kernel(**inputs):
    return _run(inputs)[0]
